# revision 40
# baseline (speedup 1.0000x reference)
"""Trainium2 Bass kernel for CapsuleLayer (dynamic routing) on 8 NeuronCores.

Problem: x[32,1152,64], W[1152,32,64,64], bias[1,1152,32,1] (zeros) ->
         out[32,32,64]
  inputs_hat = einsum('bip,icpq->bicq', x, W)
  3 rounds of routing (softmax over capsule axis, squash, agreement update).

Sharding: input-capsule axis i=1152 split over 8 cores (144 each).
W is read exactly once per core (37.7 MB bf16); only the [32,32,64]
pre-squash sum is AllReduced per routing round (split so the first,
~22us-fixed-latency collective hides under the remaining sweeps).

Layout notes:
  - Phase 1 packs FOUR i's per PSUM block: lhsT [128,128] with the even
    i-pair in cols 0:64 and the odd pair in cols 64:128 (zeros elsewhere),
    two matmuls per [128,512] block. Output partitions = 32*islot + b.
  - ih lives in SBUF for the whole kernel ([128, 36*2048] bf16): no ih HBM
    round trip.
  - The routing-coefficient application runs entirely on the TensorEngine:
    lhsT E [128, 128] has column (j, b') = coef[b', i_p, 4G+j] * d(b_p=b')
    (softmax 1/Z folded in), and 8 matmuls of N=256 per sweep consume the
    RAW SBUF ih tile - no exp-broadcast or wtt elementwise pass. Useful
    output rows per 256-column block G are (j, b) at q-slice 64j; the
    diagonal blocks are extracted once per AllReduce half. s0 (round-0
    uniform coefficients) uses the same form with a constant E0 = eye/32.
  - Per-round partial-sum layout after extraction: [128, 512], partition
    (j, b) = 32j + b holding capsules {c : c % 4 == j} as (G=c//4, q).
"""

import os
import sys

import numpy as np

for _p in (
    "/opt/trn_rl_repo",
    "/root/.axon_site",
    "/root/.axon_site/_ro/trn_rl_repo",
    "/root/.axon_site/_ro/pypackages",
):
    if os.path.isdir(_p) and _p not in sys.path:
        sys.path.append(_p)

import ml_dtypes
import concourse.bacc as bacc
import concourse.mybir as mybir
import concourse.tile as tile
from concourse.bass_utils import run_bass_kernel_spmd

F32 = mybir.dt.float32
BF16 = mybir.dt.bfloat16
AF = mybir.ActivationFunctionType
AX = mybir.AxisListType
ALU = mybir.AluOpType
BF = ml_dtypes.bfloat16

B, I, P, C, Q = 32, 1152, 64, 32, 64
N_CORES = 8
IL = I // N_CORES          # 144 input capsules per core
NQUAD = IL // 4            # 36 quads (4 i's x 32 b = 128 partitions)
CQ = C * Q                 # 2048
NUM_ROUTING = 3
SPLIT_QUAD = 26            # s0 partial AllReduced after this many quads
SPLIT_SWEEP = 24           # routing partial AllReduced after this many sweeps

CONFIG = {
    "trace": False,           # profile the run (exec_time_ns); needs ntff hook
    "trace_cores": None,      # None -> core 0 only
    "dump_s": False,          # debug: output round-1's post-AllReduce S
}

_compiled = None


def _build_kernel():
    """Build + compile the SPMD Bass module (identical program on 8 cores)."""
    nc = bacc.Bacc("TRN2", target_bir_lowering=False, debug=False,
                   num_devices=N_CORES)

    # lhsT[p, q*256 + (0:128)] = even-pair weights, (128:256) = odd-pair
    lall_d = nc.dram_tensor("lhsT", [128, NQUAD * 256], BF16,
                            kind="ExternalInput")
    # w2[q, 0] = [W(4q);W(4q+1)] stacked on p, [q, 1] = [W(4q+2);W(4q+3)]
    w_d = nc.dram_tensor("w_rhs", [NQUAD, 2, 128, CQ], BF16,
                         kind="ExternalInput")
    e0_d = nc.dram_tensor("e0", [128, 128], BF16, kind="ExternalInput")
    mask_d = nc.dram_tensor("mask_all", [128, 1024], BF16,
                            kind="ExternalInput")
    out_d = nc.dram_tensor("out", [B, CQ], F32, kind="ExternalOutput")

    rgroups = [list(range(N_CORES))]

    with tile.TileContext(nc) as tc:
        with (
            tc.tile_pool(name="ihp", bufs=1) as ih_pool,
            tc.tile_pool(name="lq", bufs=6) as lq_pool,
            tc.tile_pool(name="w", bufs=6) as w_pool,
            tc.tile_pool(name="pr", bufs=2) as pr_pool,
            tc.tile_pool(name="e4", bufs=2) as e4_pool,
            tc.tile_pool(name="v4", bufs=1) as v4_pool,
            tc.tile_pool(name="small", bufs=2) as small_pool,
            tc.tile_pool(name="acc", bufs=1) as acc_pool,
            tc.tile_pool(name="sv", bufs=1) as sv_pool,
            tc.tile_pool(name="psA", bufs=4, space="PSUM") as psA,
            tc.tile_pool(name="psB", bufs=1, space="PSUM") as psB,
            tc.tile_pool(name="dram", bufs=2, space="DRAM") as dram_pool,
        ):
            e0_t = small_pool.tile([128, 128], BF16, tag="e0")
            nc.sync.dma_start(e0_t[:], e0_d[:])
            mask_t = small_pool.tile([128, 1024], BF16, tag="mask")
            nc.sync.dma_start(mask_t[:], mask_d[:])

            b_acc = acc_pool.tile([128, NQUAD * 32], BF16, tag="bacc")
            nc.vector.memset(b_acc[:], 0.0)

            # ih for the whole core, SBUF-resident: [128=(islot,b), 36*2048]
            ihb = ih_pool.tile([128, NQUAD * CQ], BF16, tag="ihb")

            # Warm-up AllReduce: the first collective pays a large one-time
            # staging/rendezvous cost; burn it here, hidden under phase 1.
            wu_sb = small_pool.tile([32, 16], F32, tag="wu")
            nc.vector.memset(wu_sb[:], 0.0)
            wu_in = dram_pool.tile([32, 16], F32, tag="wu_in")
            wu_out = dram_pool.tile([32, 16], F32, tag="wu_out")
            nc.gpsimd.dma_start(wu_in[:], wu_sb[:])
            nc.gpsimd.collective_compute(
                "AllReduce", ALU.add,
                ins=[wu_in[:].opt()], outs=[wu_out[:].opt()],
                replica_groups=rgroups,
            )

            def flush_and_allreduce(s_ps, tag):
                """Extract the diagonal blocks of the [128, 2048] coefficient-
                matmul PSUM into [128, 512] (partition (j,b), cols (G,q)),
                then DRAM -> AllReduce. Returns the collective's output tile
                handle (read back later as [128, 512])."""
                f_sb = sv_pool.tile([128, CQ // 4], F32, tag="f_sb")
                for j in range(4):
                    src = s_ps[32 * j:32 * (j + 1), :].rearrange(
                        "b (G j2 q) -> b G j2 q", G=8, j2=4)[:, :, j, :]
                    dst = f_sb[32 * j:32 * (j + 1), :].rearrange(
                        "b (G q) -> b G q", G=8)
                    eng = nc.scalar if j % 2 == 0 else nc.vector
                    if j % 2 == 0:
                        nc.scalar.copy(dst, src)
                    else:
                        nc.vector.tensor_copy(dst, src)
                a_in = dram_pool.tile([128, CQ // 4], F32, tag="ar_in")
                a_out = dram_pool.tile([128, CQ // 4], F32, tag="ar_out")
                nc.gpsimd.dma_start(a_in[:], f_sb[:])
                nc.gpsimd.collective_compute(
                    "AllReduce", ALU.add,
                    ins=[a_in[:].opt()], outs=[a_out[:].opt()],
                    replica_groups=rgroups,
                )
                red = sv_pool.tile([128, CQ // 4], F32, tag=tag)
                nc.gpsimd.dma_start(red[:], a_out[:])
                return red

            # ---------------- Phase 1: ih = x @ W, s0 = sum_i ih / 32 -------
            def s0_matmul(q, s_ps, first, last):
                # E0 is the same for every capsule block, so one full-bank
                # N=512 matmul covers two 256-column G-blocks at once.
                for h in range(4):
                    sl = slice(512 * h, 512 * (h + 1))
                    nc.tensor.matmul(
                        s_ps[:, sl], e0_t[:],
                        ihb[:, q * CQ + 512 * h:q * CQ + 512 * (h + 1)],
                        start=first, stop=last, skip_group_check=True)

            s_ps = psB.tile([128, CQ], F32, tag="sacc")
            ar_handles = []
            for q in range(NQUAD):
                lq = lq_pool.tile([128, 256], BF16, tag="lq")
                nc.sync.dma_start(lq[:], lall_d[:, 256 * q:256 * (q + 1)])
                we = w_pool.tile([128, CQ], BF16, tag="w")
                nc.sync.dma_start(we[:], w_d[q, 0])
                # W-odd on the scalar engine's queue: a third DMA queue, and
                # crucially NOT the gpsimd queue, whose collectives would
                # stall the tail quads' loads.
                wo = w_pool.tile([128, CQ], BF16, tag="w")
                nc.scalar.dma_start(wo[:], w_d[q, 1])
                pss = []
                for blk in range(4):
                    sl = slice(512 * blk, 512 * (blk + 1))
                    ps = psA.tile([128, 512], F32)
                    pss.append(ps)
                    nc.tensor.matmul(ps[:], lq[:, 0:128], we[:, sl],
                                     start=True, stop=False)
                # s0 for the previous quad between the even/odd groups so the
                # PE never waits on the current quad's evacuation
                if q > 0:
                    qc = q - 1
                    s0_matmul(qc, s_ps,
                              first=(qc == 0 or qc == SPLIT_QUAD),
                              last=(qc == SPLIT_QUAD - 1 or qc == NQUAD - 1))
                    if qc == SPLIT_QUAD - 1:
                        ar_handles.append(flush_and_allreduce(s_ps, "ra"))
                        s_ps = psB.tile([128, CQ], F32, tag="sacc")
                for blk in range(4):
                    sl = slice(512 * blk, 512 * (blk + 1))
                    nc.tensor.matmul(pss[blk][:], lq[:, 128:256], wo[:, sl],
                                     start=False, stop=True)
                    dst = ihb[:, q * CQ + 512 * blk:q * CQ + 512 * (blk + 1)]
                    if blk in (0, 2):
                        nc.scalar.copy(dst, pss[blk][:])
                    else:
                        nc.vector.tensor_copy(dst, pss[blk][:])
            qc = NQUAD - 1
            s0_matmul(qc, s_ps, first=False, last=True)
            ar_handles.append(flush_and_allreduce(s_ps, "rb"))

            # ---------------- Routing rounds -------------------------------
            # Post-AllReduce S layout: [128, 512], partition p = 32*j + b,
            # free = (G=8, q=64) holding capsule c = 4G + j.
            C8 = C // 4     # 8 capsules per partition
            for r in range(1, NUM_ROUTING + 1):
                pa, pb = ar_handles
                S_sb = sv_pool.tile([128, CQ // 4], F32, tag="S_sb")
                nc.vector.tensor_add(S_sb[:], pa[:], pb[:])
                if CONFIG["dump_s"] and r == 1:
                    for j in range(4):
                        dst = out_d[:].rearrange(
                            "b (G j2 q) -> b G j2 q", G=8, j2=4)[:, :, j, :]
                        src = S_sb[32 * j:32 * (j + 1), :].rearrange(
                            "b (G q) -> b G q", G=8)
                        eng = nc.sync if j % 2 == 0 else nc.scalar
                        eng.dma_start(dst, src)
                    break

                # squash: v = S * sqrt(sq)/(1+sq),  sq = sum_q S^2
                S3 = S_sb[:].rearrange("b (c q) -> b c q", q=Q)
                sq = small_pool.tile([128, C8], F32, tag="sq")
                sqr = sv_pool.tile([128, CQ // 4], F32, tag="sqr")
                nc.vector.tensor_mul(sqr[:], S_sb[:], S_sb[:])
                nc.vector.reduce_sum(
                    sq[:], sqr[:].rearrange("b (c q) -> b c q", q=Q),
                    axis=AX.X)
                rt = small_pool.tile([128, C8], F32, tag="rt")
                nc.scalar.sqrt(rt[:], sq[:])
                onep = small_pool.tile([128, C8], F32, tag="onep")
                nc.vector.tensor_scalar_add(onep[:], sq[:], 1.0)
                rden = small_pool.tile([128, C8], F32, tag="rden")
                nc.vector.reciprocal(rden[:], onep[:])
                scale = small_pool.tile([128, C8], F32, tag="scale")
                nc.vector.tensor_mul(scale[:], rt[:], rden[:])
                scale_b = scale[:].unsqueeze(-1).broadcast_to((128, C8, Q))

                if r == NUM_ROUTING:
                    # v overwrites S_sb in place; reorder (j,b)/(G,q) ->
                    # [B, (c=4G+j, q)] during the output DMA
                    nc.vector.tensor_mul(S3, S3, scale_b)
                    for j in range(4):
                        dst = out_d[:].rearrange(
                            "b (G j2 q) -> b G j2 q", G=8, j2=4)[:, :, j, :]
                        src = S_sb[32 * j:32 * (j + 1), :].rearrange(
                            "b (G q) -> b G q", G=8)
                        eng = nc.sync if j % 2 == 0 else nc.scalar
                        eng.dma_start(dst, src)
                    break

                # v at bf16; v4 [128, 2048] holds the c-major [32b, (c,q)]
                # v-rows replicated in all 4 islot groups: 4 strided DMAs
                # de-interleave (j,b)/(G,q) -> slot 0, then 3 slot copies.
                v_c = sv_pool.tile([128, CQ // 4], BF16, tag="v_c")
                nc.vector.tensor_mul(
                    v_c[:].rearrange("b (c q) -> b c q", q=Q), S3, scale_b)
                v4 = v4_pool.tile([128, CQ], BF16, tag="v4")
                for j in range(4):
                    dst = v4[0:32, :].rearrange(
                        "b (G j2 q) -> b G j2 q", G=8, j2=4)[:, :, j, :]
                    src = v_c[32 * j:32 * (j + 1), :].rearrange(
                        "b (G q) -> b G q", G=8)
                    eng = nc.gpsimd if j % 2 == 0 else nc.scalar
                    eng.dma_start(dst, src)
                for g in range(1, 4):
                    eng = nc.gpsimd if g % 2 == 0 else nc.scalar
                    eng.dma_start(v4[32 * g:32 * (g + 1), :], v4[0:32, :])

                ar_handles = []
                s_ps = psB.tile([128, CQ], F32, tag="sacc")
                for s in range(NQUAD):
                    if s == SPLIT_SWEEP:
                        ar_handles.append(flush_and_allreduce(s_ps, "ra"))
                        s_ps = psB.tile([128, CQ], F32, tag="sacc")
                    first, last_s = (s == 0 or s == SPLIT_SWEEP), \
                        (s == SPLIT_SWEEP - 1 or s == NQUAD - 1)
                    it = ihb[:, s * CQ:(s + 1) * CQ]
                    # logits: dlog[(i,b), c] = sum_q ih*v: DVE 2x
                    # tensor_tensor product + in-place pair-add tree + a
                    # short fp32 reduce
                    pr = pr_pool.tile([128, CQ], BF16, tag="pr")
                    nc.vector.tensor_mul(pr[:], it, v4[:])
                    pr3 = pr[:].rearrange("p (c q) -> p c q", q=Q)
                    nc.vector.tensor_add(
                        pr3[:, :, 0:32], pr3[:, :, 0:32], pr3[:, :, 32:64])
                    nc.vector.tensor_add(
                        pr3[:, :, 0:16], pr3[:, :, 0:16], pr3[:, :, 16:32])
                    dlog = small_pool.tile([128, C], F32, tag="dlog")
                    nc.vector.reduce_sum(dlog[:], pr3[:, :, 0:16], axis=AX.X)
                    bsl = b_acc[:, 32 * s:32 * (s + 1)]
                    nc.vector.tensor_add(bsl, bsl, dlog[:])
                    # softmax over c: exp on ACT (Z via accumulator); the
                    # coefficient matrix E[p, (G,j,b')] = mask * exp * 1/Z
                    # feeds the TensorEngine directly - no elementwise pass
                    # over the [128, 2048] ih tile.
                    e = small_pool.tile([128, C], BF16, tag="e")
                    z = small_pool.tile([128, 1], F32, tag="z")
                    nc.scalar.activation(e[:], bsl, AF.Exp, accum_out=z[:])
                    rz = small_pool.tile([128, 1], F32, tag="rz")
                    nc.vector.reciprocal(rz[:], z[:])
                    e4 = e4_pool.tile([128, 1024], BF16, tag="e4")
                    nc.scalar.activation(
                        e4[:].rearrange("p (G j b) -> p G j b", G=8, j=4),
                        e[:].rearrange("p (G j) -> p G j", j=4)
                            .unsqueeze(-1).broadcast_to((128, 8, 4, 32)),
                        AF.Copy, scale=rz[:])
                    nc.vector.tensor_mul(e4[:], mask_t[:], e4[:])
                    for g in range(8):
                        sl = slice(256 * g, 256 * (g + 1))
                        nc.tensor.matmul(
                            s_ps[:, sl], e4[:, 128 * g:128 * (g + 1)],
                            it[:, sl], start=first and g % 2 == 0,
                            stop=last_s, skip_group_check=True)
                ar_handles.append(flush_and_allreduce(s_ps, "rb"))

    nc.compile()
    return nc


def _prep_core_inputs(x, W):
    """Host-side shard + repack for one call. Returns list of in_maps."""
    xs_all = np.ascontiguousarray(x)          # [B, I, P]
    in_maps = []
    eye4 = np.tile(np.eye(32, dtype=np.float32), (4, 1))   # [128, 32]
    e0 = np.tile(eye4 / C, (1, 4)).astype(BF)              # [128, 128]
    mask_all = np.tile(eye4, (1, 32)).astype(BF)           # [128, 1024]
    for k in range(N_CORES):
        xs = xs_all[:, k * IL:(k + 1) * IL, :]          # [B, IL, P]
        # lhsT per quad: [128, 256]; even half cols 0:128, odd cols 128:256.
        #   even: col 32j+b (j=0,1) <- xs[b, 4Q+j, p] at partitions 64j+p
        #   odd:  col 64+32j+b      <- xs[b, 4Q+2+j, p] at partitions 64j+p
        xt = xs.transpose(1, 2, 0).reshape(NQUAD, 4, P, B)  # [Q, j, p, b]
        lhsT = np.zeros((NQUAD, 128, 256), np.float32)
        lhsT[:, 0:64, 0:32] = xt[:, 0]
        lhsT[:, 64:128, 32:64] = xt[:, 1]
        lhsT[:, 0:64, 128 + 64:128 + 96] = xt[:, 2]
        lhsT[:, 64:128, 128 + 96:128 + 128] = xt[:, 3]
        lall = np.ascontiguousarray(
            lhsT.astype(BF).transpose(1, 0, 2)).reshape(128, -1)
        Ws = W[k * IL:(k + 1) * IL]                      # [IL, C, P, Q]
        # [p, (c q)] per i, stacked in pairs of two i's on the p axis
        w_rhs = np.ascontiguousarray(
            Ws.reshape(NQUAD, 2, 2, C, P, Q).transpose(0, 1, 2, 4, 3, 5)
        ).reshape(NQUAD, 2, 128, CQ).astype(BF)
        in_maps.append({"lhsT": lall, "w_rhs": np.ascontiguousarray(w_rhs),
                        "e0": e0, "mask_all": mask_all})
    return in_maps


def _host_reference(x, W, bias):
    """Exact numpy fallback (used only if bias != 0, which the problem's
    input spec says cannot happen; the device kernel assumes uniform
    round-0 routing coefficients)."""
    ih = np.einsum("bip,icpq->bicq", x, W)
    b = bias.astype(np.float64)
    out = None
    for r in range(NUM_ROUTING):
        e = np.exp(b - b.max(axis=2, keepdims=True))
        c = e / e.sum(axis=2, keepdims=True)
        s = (c * ih).sum(axis=1, keepdims=True)
        sq = np.sum(s * s, axis=-1, keepdims=True)
        out = s * (sq / (1.0 + sq) / np.sqrt(sq))
        if r != NUM_ROUTING - 1:
            b = b + np.sum(ih * out, axis=-1, keepdims=True)
    return out.reshape(B, C, Q).astype(np.float32)


def kernel(x, W, bias):
    global _compiled
    x = np.asarray(x, dtype=np.float32)
    W = np.asarray(W, dtype=np.float32)
    bias = np.asarray(bias, dtype=np.float32)
    if np.any(bias):
        return _host_reference(x, W, bias)

    if _compiled is None:
        _compiled = _build_kernel()
    nc = _compiled

    in_maps = _prep_core_inputs(x, W)
    res = run_bass_kernel_spmd(
        nc, in_maps, list(range(N_CORES)),
        trace=CONFIG["trace"], trace_cores=CONFIG["trace_cores"],
    )
    kernel.last_results = res
    out = res.results[0]["out"].reshape(B, C, Q)
    return out


# revision 41
# speedup vs baseline: 1.0656x; 1.0656x over previous
"""Trainium2 Bass kernel for CapsuleLayer (dynamic routing) on 8 NeuronCores.

Problem: x[32,1152,64], W[1152,32,64,64], bias[1,1152,32,1] (zeros) ->
         out[32,32,64]
  inputs_hat = einsum('bip,icpq->bicq', x, W)
  3 rounds of routing (softmax over capsule axis, squash, agreement update).

Sharding: input-capsule axis i=1152 split over 8 cores (144 each).
W is read exactly once per core (37.7 MB bf16); only the [32,32,64]
pre-squash sum is AllReduced per routing round (split so the first,
~22us-fixed-latency collective hides under the remaining sweeps).

Layout notes:
  - Phase 1 packs FOUR i's per PSUM block: lhsT [128,128] with the even
    i-pair in cols 0:64 and the odd pair in cols 64:128 (zeros elsewhere),
    two matmuls per [128,512] block. Output partitions = 32*islot + b.
  - ih lives in SBUF for the whole kernel ([128, 36*2048] bf16): no ih HBM
    round trip.
  - The routing-coefficient application runs entirely on the TensorEngine:
    lhsT E [128, 128] has column (j, b') = coef[b', i_p, 4G+j] * d(b_p=b')
    (softmax 1/Z folded in), and 8 matmuls of N=256 per sweep consume the
    RAW SBUF ih tile - no exp-broadcast or wtt elementwise pass. Useful
    output rows per 256-column block G are (j, b) at q-slice 64j; the
    diagonal blocks are extracted once per AllReduce half. s0 (round-0
    uniform coefficients) uses the same form with a constant E0 = eye/32.
  - Per-round partial-sum layout after extraction: [128, 512], partition
    (j, b) = 32j + b holding capsules {c : c % 4 == j} as (G=c//4, q).
"""

import os
import sys

import numpy as np

for _p in (
    "/opt/trn_rl_repo",
    "/root/.axon_site",
    "/root/.axon_site/_ro/trn_rl_repo",
    "/root/.axon_site/_ro/pypackages",
):
    if os.path.isdir(_p) and _p not in sys.path:
        sys.path.append(_p)

import ml_dtypes
import concourse.bacc as bacc
import concourse.mybir as mybir
import concourse.tile as tile
from concourse.bass_utils import run_bass_kernel_spmd

F32 = mybir.dt.float32
BF16 = mybir.dt.bfloat16
AF = mybir.ActivationFunctionType
AX = mybir.AxisListType
ALU = mybir.AluOpType
BF = ml_dtypes.bfloat16

B, I, P, C, Q = 32, 1152, 64, 32, 64
N_CORES = 8
IL = I // N_CORES          # 144 input capsules per core
NQUAD = IL // 4            # 36 quads (4 i's x 32 b = 128 partitions)
CQ = C * Q                 # 2048
NUM_ROUTING = 3
SPLIT_QUAD = 26            # s0 partial AllReduced after this many quads
SPLIT_SWEEP = 24           # routing partial AllReduced after this many sweeps

CONFIG = {
    "trace": False,           # profile the run (exec_time_ns); needs ntff hook
    "trace_cores": None,      # None -> core 0 only
    "dump_s": False,          # debug: output round-1's post-AllReduce S
}

_compiled = None


def _build_kernel():
    """Build + compile the SPMD Bass module (identical program on 8 cores)."""
    nc = bacc.Bacc("TRN2", target_bir_lowering=False, debug=False,
                   num_devices=N_CORES)

    # lhsT[p, q*256 + (0:128)] = even-pair weights, (128:256) = odd-pair
    lall_d = nc.dram_tensor("lhsT", [128, NQUAD * 256], BF16,
                            kind="ExternalInput")
    # w2[q, 0] = [W(4q);W(4q+1)] stacked on p, [q, 1] = [W(4q+2);W(4q+3)]
    w_d = nc.dram_tensor("w_rhs", [NQUAD, 2, 128, CQ], BF16,
                         kind="ExternalInput")
    e0_d = nc.dram_tensor("e0", [128, 128], BF16, kind="ExternalInput")
    mask_d = nc.dram_tensor("mask_all", [128, 1024], BF16,
                            kind="ExternalInput")
    out_d = nc.dram_tensor("out", [B, CQ], F32, kind="ExternalOutput")

    rgroups = [list(range(N_CORES))]

    with tile.TileContext(nc) as tc:
        with (
            tc.tile_pool(name="ihp", bufs=1) as ih_pool,
            tc.tile_pool(name="lq", bufs=6) as lq_pool,
            tc.tile_pool(name="w", bufs=6) as w_pool,
            tc.tile_pool(name="pr", bufs=2) as pr_pool,
            tc.tile_pool(name="e4", bufs=2) as e4_pool,
            tc.tile_pool(name="v4", bufs=1) as v4_pool,
            tc.tile_pool(name="small", bufs=2) as small_pool,
            tc.tile_pool(name="acc", bufs=1) as acc_pool,
            tc.tile_pool(name="sv", bufs=1) as sv_pool,
            tc.tile_pool(name="psA", bufs=4, space="PSUM") as psA,
            tc.tile_pool(name="psB", bufs=1, space="PSUM") as psB,
            tc.tile_pool(name="dram", bufs=2, space="DRAM") as dram_pool,
        ):
            e0_t = small_pool.tile([128, 128], BF16, tag="e0")
            nc.sync.dma_start(e0_t[:], e0_d[:])
            mask_t = small_pool.tile([128, 1024], BF16, tag="mask")
            nc.sync.dma_start(mask_t[:], mask_d[:])

            b_acc = acc_pool.tile([128, NQUAD * 32], BF16, tag="bacc")
            nc.vector.memset(b_acc[:], 0.0)

            # ih for the whole core, SBUF-resident: [128=(islot,b), 36*2048]
            ihb = ih_pool.tile([128, NQUAD * CQ], BF16, tag="ihb")

            # Warm-up AllReduce: the first collective pays a large one-time
            # staging/rendezvous cost; burn it here, hidden under phase 1.
            wu_sb = small_pool.tile([32, 16], F32, tag="wu")
            nc.vector.memset(wu_sb[:], 0.0)
            wu_in = dram_pool.tile([32, 16], F32, tag="wu_in")
            wu_out = dram_pool.tile([32, 16], F32, tag="wu_out")
            nc.gpsimd.dma_start(wu_in[:], wu_sb[:])
            nc.gpsimd.collective_compute(
                "AllReduce", ALU.add,
                ins=[wu_in[:].opt()], outs=[wu_out[:].opt()],
                replica_groups=rgroups,
            )

            def flush_and_allreduce(s_ps, tag):
                """Extract the diagonal blocks of the [128, 2048] coefficient-
                matmul PSUM into [128, 512] (partition (j,b), cols (G,q)),
                then DRAM -> AllReduce. Returns the collective's output tile
                handle (read back later as [128, 512])."""
                f_sb = sv_pool.tile([128, CQ // 4], F32, tag="f_sb")
                for j in range(4):
                    src = s_ps[32 * j:32 * (j + 1), :].rearrange(
                        "b (G j2 q) -> b G j2 q", G=8, j2=4)[:, :, j, :]
                    dst = f_sb[32 * j:32 * (j + 1), :].rearrange(
                        "b (G q) -> b G q", G=8)
                    eng = nc.scalar if j % 2 == 0 else nc.vector
                    if j % 2 == 0:
                        nc.scalar.copy(dst, src)
                    else:
                        nc.vector.tensor_copy(dst, src)
                a_in = dram_pool.tile([128, CQ // 4], F32, tag="ar_in")
                a_out = dram_pool.tile([128, CQ // 4], F32, tag="ar_out")
                nc.gpsimd.dma_start(a_in[:], f_sb[:])
                nc.gpsimd.collective_compute(
                    "AllReduce", ALU.add,
                    ins=[a_in[:].opt()], outs=[a_out[:].opt()],
                    replica_groups=rgroups,
                )
                red = sv_pool.tile([128, CQ // 4], F32, tag=tag)
                nc.gpsimd.dma_start(red[:], a_out[:])
                return red

            # ---------------- Phase 1: ih = x @ W, s0 = sum_i ih / 32 -------
            def s0_matmul(q, s_ps, first, last):
                # start=True zeroes the WHOLE 2KB PSUM bank, not just the
                # 256-column region: only the first (even-g) matmul of each
                # bank may start, and its zero covers the odd-g half too.
                for g in range(8):
                    sl = slice(256 * g, 256 * (g + 1))
                    nc.tensor.matmul(
                        s_ps[:, sl], e0_t[:],
                        ihb[:, q * CQ + 256 * g:q * CQ + 256 * (g + 1)],
                        start=first and g % 2 == 0, stop=last,
                        skip_group_check=True)

            s_ps = psB.tile([128, CQ], F32, tag="sacc")
            ar_handles = []
            for q in range(NQUAD):
                lq = lq_pool.tile([128, 256], BF16, tag="lq")
                nc.sync.dma_start(lq[:], lall_d[:, 256 * q:256 * (q + 1)])
                we = w_pool.tile([128, CQ], BF16, tag="w")
                nc.sync.dma_start(we[:], w_d[q, 0])
                # W-odd on the scalar engine's queue: a third DMA queue, and
                # crucially NOT the gpsimd queue, whose collectives would
                # stall the tail quads' loads.
                wo = w_pool.tile([128, CQ], BF16, tag="w")
                nc.scalar.dma_start(wo[:], w_d[q, 1])
                pss = []
                for blk in range(4):
                    sl = slice(512 * blk, 512 * (blk + 1))
                    ps = psA.tile([128, 512], F32)
                    pss.append(ps)
                    nc.tensor.matmul(ps[:], lq[:, 0:128], we[:, sl],
                                     start=True, stop=False)
                # s0 for the previous quad between the even/odd groups so the
                # PE never waits on the current quad's evacuation
                if q > 0:
                    qc = q - 1
                    s0_matmul(qc, s_ps,
                              first=(qc == 0 or qc == SPLIT_QUAD),
                              last=(qc == SPLIT_QUAD - 1 or qc == NQUAD - 1))
                    if qc == SPLIT_QUAD - 1:
                        ar_handles.append(flush_and_allreduce(s_ps, "ra"))
                        s_ps = psB.tile([128, CQ], F32, tag="sacc")
                for blk in range(4):
                    sl = slice(512 * blk, 512 * (blk + 1))
                    nc.tensor.matmul(pss[blk][:], lq[:, 128:256], wo[:, sl],
                                     start=False, stop=True)
                    dst = ihb[:, q * CQ + 512 * blk:q * CQ + 512 * (blk + 1)]
                    if blk in (0, 2):
                        nc.scalar.copy(dst, pss[blk][:])
                    else:
                        nc.vector.tensor_copy(dst, pss[blk][:])
            qc = NQUAD - 1
            s0_matmul(qc, s_ps, first=False, last=True)
            ar_handles.append(flush_and_allreduce(s_ps, "rb"))

            # ---------------- Routing rounds -------------------------------
            # Post-AllReduce S layout: [128, 512], partition p = 32*j + b,
            # free = (G=8, q=64) holding capsule c = 4G + j.
            C8 = C // 4     # 8 capsules per partition
            for r in range(1, NUM_ROUTING + 1):
                pa, pb = ar_handles
                S_sb = sv_pool.tile([128, CQ // 4], F32, tag="S_sb")
                nc.vector.tensor_add(S_sb[:], pa[:], pb[:])
                if CONFIG["dump_s"] and r == 1:
                    for j in range(4):
                        dst = out_d[:].rearrange(
                            "b (G j2 q) -> b G j2 q", G=8, j2=4)[:, :, j, :]
                        src = S_sb[32 * j:32 * (j + 1), :].rearrange(
                            "b (G q) -> b G q", G=8)
                        eng = nc.sync if j % 2 == 0 else nc.scalar
                        eng.dma_start(dst, src)
                    break

                # squash: v = S * sqrt(sq)/(1+sq),  sq = sum_q S^2
                S3 = S_sb[:].rearrange("b (c q) -> b c q", q=Q)
                sq = small_pool.tile([128, C8], F32, tag="sq")
                sqr = sv_pool.tile([128, CQ // 4], F32, tag="sqr")
                nc.vector.tensor_mul(sqr[:], S_sb[:], S_sb[:])
                nc.vector.reduce_sum(
                    sq[:], sqr[:].rearrange("b (c q) -> b c q", q=Q),
                    axis=AX.X)
                rt = small_pool.tile([128, C8], F32, tag="rt")
                nc.scalar.sqrt(rt[:], sq[:])
                onep = small_pool.tile([128, C8], F32, tag="onep")
                nc.vector.tensor_scalar_add(onep[:], sq[:], 1.0)
                rden = small_pool.tile([128, C8], F32, tag="rden")
                nc.vector.reciprocal(rden[:], onep[:])
                scale = small_pool.tile([128, C8], F32, tag="scale")
                nc.vector.tensor_mul(scale[:], rt[:], rden[:])
                scale_b = scale[:].unsqueeze(-1).broadcast_to((128, C8, Q))

                if r == NUM_ROUTING:
                    # v overwrites S_sb in place; reorder (j,b)/(G,q) ->
                    # [B, (c=4G+j, q)] during the output DMA
                    nc.vector.tensor_mul(S3, S3, scale_b)
                    for j in range(4):
                        dst = out_d[:].rearrange(
                            "b (G j2 q) -> b G j2 q", G=8, j2=4)[:, :, j, :]
                        src = S_sb[32 * j:32 * (j + 1), :].rearrange(
                            "b (G q) -> b G q", G=8)
                        eng = nc.sync if j % 2 == 0 else nc.scalar
                        eng.dma_start(dst, src)
                    break

                # v at bf16; v4 [128, 2048] holds the c-major [32b, (c,q)]
                # v-rows replicated in all 4 islot groups: 4 strided DMAs
                # de-interleave (j,b)/(G,q) -> slot 0, then 3 slot copies.
                v_c = sv_pool.tile([128, CQ // 4], BF16, tag="v_c")
                nc.vector.tensor_mul(
                    v_c[:].rearrange("b (c q) -> b c q", q=Q), S3, scale_b)
                v4 = v4_pool.tile([128, CQ], BF16, tag="v4")
                for j in range(4):
                    dst = v4[0:32, :].rearrange(
                        "b (G j2 q) -> b G j2 q", G=8, j2=4)[:, :, j, :]
                    src = v_c[32 * j:32 * (j + 1), :].rearrange(
                        "b (G q) -> b G q", G=8)
                    eng = nc.gpsimd if j % 2 == 0 else nc.scalar
                    eng.dma_start(dst, src)
                for g in range(1, 4):
                    eng = nc.gpsimd if g % 2 == 0 else nc.scalar
                    eng.dma_start(v4[32 * g:32 * (g + 1), :], v4[0:32, :])

                ar_handles = []
                s_ps = psB.tile([128, CQ], F32, tag="sacc")
                for s in range(NQUAD):
                    if s == SPLIT_SWEEP:
                        ar_handles.append(flush_and_allreduce(s_ps, "ra"))
                        s_ps = psB.tile([128, CQ], F32, tag="sacc")
                    first, last_s = (s == 0 or s == SPLIT_SWEEP), \
                        (s == SPLIT_SWEEP - 1 or s == NQUAD - 1)
                    it = ihb[:, s * CQ:(s + 1) * CQ]
                    # logits: dlog[(i,b), c] = sum_q ih*v: DVE 2x
                    # tensor_tensor product + in-place pair-add tree + a
                    # short fp32 reduce
                    pr = pr_pool.tile([128, CQ], BF16, tag="pr")
                    nc.vector.tensor_mul(pr[:], it, v4[:])
                    pr3 = pr[:].rearrange("p (c q) -> p c q", q=Q)
                    nc.vector.tensor_add(
                        pr3[:, :, 0:32], pr3[:, :, 0:32], pr3[:, :, 32:64])
                    nc.vector.tensor_add(
                        pr3[:, :, 0:16], pr3[:, :, 0:16], pr3[:, :, 16:32])
                    dlog = small_pool.tile([128, C], F32, tag="dlog")
                    nc.vector.reduce_sum(dlog[:], pr3[:, :, 0:16], axis=AX.X)
                    bsl = b_acc[:, 32 * s:32 * (s + 1)]
                    nc.vector.tensor_add(bsl, bsl, dlog[:])
                    # softmax over c: exp on ACT (Z via accumulator); the
                    # coefficient matrix E[p, (G,j,b')] = mask * exp * 1/Z
                    # feeds the TensorEngine directly - no elementwise pass
                    # over the [128, 2048] ih tile.
                    e = small_pool.tile([128, C], BF16, tag="e")
                    z = small_pool.tile([128, 1], F32, tag="z")
                    nc.scalar.activation(e[:], bsl, AF.Exp, accum_out=z[:])
                    rz = small_pool.tile([128, 1], F32, tag="rz")
                    nc.vector.reciprocal(rz[:], z[:])
                    e4 = e4_pool.tile([128, 1024], BF16, tag="e4")
                    nc.scalar.activation(
                        e4[:].rearrange("p (G j b) -> p G j b", G=8, j=4),
                        e[:].rearrange("p (G j) -> p G j", j=4)
                            .unsqueeze(-1).broadcast_to((128, 8, 4, 32)),
                        AF.Copy, scale=rz[:])
                    nc.vector.tensor_mul(e4[:], mask_t[:], e4[:])
                    for g in range(8):
                        sl = slice(256 * g, 256 * (g + 1))
                        nc.tensor.matmul(
                            s_ps[:, sl], e4[:, 128 * g:128 * (g + 1)],
                            it[:, sl], start=first and g % 2 == 0,
                            stop=last_s, skip_group_check=True)
                ar_handles.append(flush_and_allreduce(s_ps, "rb"))

    nc.compile()
    return nc


def _prep_core_inputs(x, W):
    """Host-side shard + repack for one call. Returns list of in_maps."""
    xs_all = np.ascontiguousarray(x)          # [B, I, P]
    in_maps = []
    eye4 = np.tile(np.eye(32, dtype=np.float32), (4, 1))   # [128, 32]
    e0 = np.tile(eye4 / C, (1, 4)).astype(BF)              # [128, 128]
    mask_all = np.tile(eye4, (1, 32)).astype(BF)           # [128, 1024]
    for k in range(N_CORES):
        xs = xs_all[:, k * IL:(k + 1) * IL, :]          # [B, IL, P]
        # lhsT per quad: [128, 256]; even half cols 0:128, odd cols 128:256.
        #   even: col 32j+b (j=0,1) <- xs[b, 4Q+j, p] at partitions 64j+p
        #   odd:  col 64+32j+b      <- xs[b, 4Q+2+j, p] at partitions 64j+p
        xt = xs.transpose(1, 2, 0).reshape(NQUAD, 4, P, B)  # [Q, j, p, b]
        lhsT = np.zeros((NQUAD, 128, 256), np.float32)
        lhsT[:, 0:64, 0:32] = xt[:, 0]
        lhsT[:, 64:128, 32:64] = xt[:, 1]
        lhsT[:, 0:64, 128 + 64:128 + 96] = xt[:, 2]
        lhsT[:, 64:128, 128 + 96:128 + 128] = xt[:, 3]
        lall = np.ascontiguousarray(
            lhsT.astype(BF).transpose(1, 0, 2)).reshape(128, -1)
        Ws = W[k * IL:(k + 1) * IL]                      # [IL, C, P, Q]
        # [p, (c q)] per i, stacked in pairs of two i's on the p axis
        w_rhs = np.ascontiguousarray(
            Ws.reshape(NQUAD, 2, 2, C, P, Q).transpose(0, 1, 2, 4, 3, 5)
        ).reshape(NQUAD, 2, 128, CQ).astype(BF)
        in_maps.append({"lhsT": lall, "w_rhs": np.ascontiguousarray(w_rhs),
                        "e0": e0, "mask_all": mask_all})
    return in_maps


def _host_reference(x, W, bias):
    """Exact numpy fallback (used only if bias != 0, which the problem's
    input spec says cannot happen; the device kernel assumes uniform
    round-0 routing coefficients)."""
    ih = np.einsum("bip,icpq->bicq", x, W)
    b = bias.astype(np.float64)
    out = None
    for r in range(NUM_ROUTING):
        e = np.exp(b - b.max(axis=2, keepdims=True))
        c = e / e.sum(axis=2, keepdims=True)
        s = (c * ih).sum(axis=1, keepdims=True)
        sq = np.sum(s * s, axis=-1, keepdims=True)
        out = s * (sq / (1.0 + sq) / np.sqrt(sq))
        if r != NUM_ROUTING - 1:
            b = b + np.sum(ih * out, axis=-1, keepdims=True)
    return out.reshape(B, C, Q).astype(np.float32)


def kernel(x, W, bias):
    global _compiled
    x = np.asarray(x, dtype=np.float32)
    W = np.asarray(W, dtype=np.float32)
    bias = np.asarray(bias, dtype=np.float32)
    if np.any(bias):
        return _host_reference(x, W, bias)

    if _compiled is None:
        _compiled = _build_kernel()
    nc = _compiled

    in_maps = _prep_core_inputs(x, W)
    res = run_bass_kernel_spmd(
        nc, in_maps, list(range(N_CORES)),
        trace=CONFIG["trace"], trace_cores=CONFIG["trace_cores"],
    )
    kernel.last_results = res
    out = res.results[0]["out"].reshape(B, C, Q)
    return out
